# revision 26
# baseline (speedup 1.0000x reference)
"""Trainium2 Bass kernel for nn_Encoder_74182675137046.

Reference computation (per image of 1024x1024 complex pixels):
    feats = [norm_row, norm_col, x0, x1]  per pixel     [N, 4]
    h   = relu((feats @ W1 + b1) @ W2 + b2)             [N, 128]
    out = h @ W3 + b3                                   [N, 128]
    result = (w * out).sum(0) / w.sum()                 [128]
with w = (x0 != 0), and norm_row/col normalized by masked min/max.

Algebraic folding (exact):
    fc1+fc2 fold:  h_pre = feats @ W12 + b12,  W12 = W1@W2, b12 = b1@W2 + b2
    pool/fc3 swap: (w*out).sum = (sum_p w_p*relu(h_pre_p)) @ W3 + w.sum()*b3
So the device only computes S = sum_p relu(h_pre_p)  (a [128] vector per
core); the tiny [128]x[128,128] tail runs on host in float64.

Device design (per core, 128 image rows = 131072 points):
  - bf16 matmuls (1 cycle/col on PE vs ~2.8 for f32r), K=8 rows per
    group: [x0, x1, x0, x1, nc, 1, nc, 1] paired with hi/lo-split
    weights [v2h, v3h, v2l, v3l, v1h, bh, v1l, bl] so effective weights
    carry ~fp32 precision (only feature values are bf16-rounded).  The
    norm_row term + b12 fold into the per-image-row bias column (bh/bl).
  - fp32 PSUM [128, 8, 512] (TRN2 matmul cannot write 16-bit PSUM).
    Round r covers image rows 4r..4r+3: bank b <- (row 4r + b%4,
    col-half b//4); the two col-halves of a row share one lhsT block.
    Four tile_position row groups run concurrently per wave.
  - Consumers per round (4096 els/partition): ScalarE relu+accum on
    banks 0-3 (FD 2048), VectorE max(0,.)+accum on banks 4-7 (FD 2048),
    running concurrently on disjoint banks; 32 rounds.
  - Accumulators land in red[128, 64]; final reduce_sum + DMA returns
    the per-core S vector; the tail (@ W3, /wsum, +b3) runs on host.
"""

import numpy as np
import ml_dtypes

import concourse.bass as bass
import concourse.tile as tile
from concourse import mybir
from concourse.bass_utils import run_bass_kernel_spmd
from concourse.tile_rust import add_dep_helper

H = 1024
W = 1024
D = 128
N_CORES = 8
ROWS_PER_CORE = H // N_CORES          # 128
NROUNDS = 32                          # rounds per core; 4 image rows each
NB = 8                                # PSUM banks (512 fp32 each)

F32 = mybir.dt.float32
BF16 = mybir.dt.bfloat16
BF16_NP = ml_dtypes.bfloat16

TRACE = False
LAST_RESULT = None

_NC_CACHE = None


def _build_bass():
    """Build the SPMD Bass program (same program on all 8 cores)."""
    global _NC_CACHE
    if _NC_CACHE is not None:
        return _NC_CACHE

    nc = bass.Bass()

    xd = nc.dram_tensor("xd", [NROUNDS, 4, 8, 1024], BF16,
                        kind="ExternalInput")
    lwt = nc.dram_tensor("lwt", [4, 8, 32 * 128], BF16,
                         kind="ExternalInput")
    outs = nc.dram_tensor("outs", [128, 1], F32, kind="ExternalOutput")

    with tile.TileContext(nc) as tc:
        with (
            tc.tile_pool(name="singles", bufs=1) as singles,
            tc.tile_pool(name="psall", bufs=1, space="PSUM") as psall,
        ):
            lw_t = singles.tile([128, 32 * 128], BF16)
            rhs_t = singles.tile([128, NROUNDS, 1024], BF16)
            red = singles.tile([128, 2 * NROUNDS], F32)
            outs_t = singles.tile([128, 1], F32)
            tiny_a = singles.tile([128, 1], F32)
            tiny_v = singles.tile([128, 1], F32)
            tiny_v2 = singles.tile([128, 1], F32)
            # Ping-pong SBUF scratch for consumer outputs: avoids the
            # in-place PSUM write's same-engine distance-1 hazard (which
            # costs a second, ISA-illegal sem wait on the consumers).
            scr_a = singles.tile([128, 2, 4 * 512], BF16)
            scr_v = singles.tile([128, 2, 4 * 512], BF16)
            ps = psall.tile([128, NB, 512], F32)

            # DMA out APs must be a contiguous partition block with the
            # free run inside one partition: multi-level partition views
            # get flattened across partition boundaries by the AP
            # optimizer and clobber neighboring tiles.
            lw_dmas = [
                nc.gpsimd.dma_start(out=lw_t[32 * g:32 * g + 8, :],
                                    in_=lwt[g])
                for g in range(4)
            ]
            # Per-round rhs rows (x0, x1 duplicated + const rows) on the
            # sync HWDGE queues; one DMA per (round, group).
            x_dmas = [
                [nc.sync.dma_start(out=rhs_t[32 * g:32 * g + 8, r, :],
                                   in_=xd[r, g])
                 for g in range(4)]
                for r in range(NROUNDS)
            ]

            def obs_mm(src_ap, tp=0):
                """Tiny observer matmul: natural data dep on src_ap's
                producer becomes this instruction's single sem wait; later
                PE instructions with the same dep get it elided.  Writes a
                scratch PSUM cell in bank 0 (overwritten by the round's
                real bank-0 matmul before any consumer reads it)."""
                return nc.tensor.matmul(
                    ps[0:1, 0, 0:1], src_ap, src_ap,
                    start=True, stop=True, tile_position=(tp, 0),
                )

            last_mms = None
            cons = []
            prev_a = prev_v = None
            for r in range(NROUNDS):
                # PE-side observers: each carries exactly one cross-engine
                # sem wait (matmul ISA slots allow only one); the real
                # matmuls then run wait-free, their deps covered by
                # same-engine order behind the observers.
                obs = []
                if prev_a is not None:
                    # Reads cons_a(r-1)'s scratch output (bf16 -> the obs
                    # matmul stays bf16; fp32 obs would interleave
                    # fp32-HIGH with FWL bf16 matmuls, a known HW-hang
                    # combination) -> ACT sem wait, which also covers the
                    # scratch cell's WAR vs cons_a(r-1).
                    obs.append(obs_mm(scr_a[0:1, (r - 1) % 2, 0:1]))
                if prev_v is not None:
                    obs.append(obs_mm(scr_v[0:1, (r - 1) % 2, 0:1]))
                if r == 0:
                    for g in range(4):
                        obs.append(obs_mm(lw_t[32 * g:32 * g + 1, 0:1],
                                          tp=32 * g))
                for g in range(4):
                    obs.append(obs_mm(rhs_t[32 * g:32 * g + 1, r, 0:1],
                                      tp=32 * g))
                for o1, o2 in zip(obs, obs[1:]):
                    add_dep_helper(o2.ins, o1.ins, reason="obs chain")

                mms = []
                for b in range(NB):
                    g = b % 4
                    half = b // 4
                    mm = nc.tensor.matmul(
                        ps[:, b, :],
                        lw_t[32 * g:32 * g + 8, 128 * r:128 * (r + 1)],
                        rhs_t[32 * g:32 * g + 8, r,
                              512 * half:512 * half + 512],
                        start=True, stop=True,
                        tile_position=(32 * g, 0),
                    )
                    add_dep_helper(mm.ins, obs[-1].ins, reason="mm after obs")
                    mms.append(mm)
                last_mms = mms

                # Carriers: tiny same-engine ops absorbing the accumulator
                # completion-sem of the previous consumer, so each consumer
                # keeps a single (PE) sem wait.
                if prev_a is not None:
                    ca = nc.scalar.activation(
                        out=tiny_a[:], in_=red[:, 0:1],
                        func=mybir.ActivationFunctionType.Relu,
                    )
                    add_dep_helper(ca.ins, prev_a.ins, reason="ACT WAW")
                if prev_v is not None:
                    cv = nc.vector.tensor_scalar(
                        out=tiny_v[:], in0=red[:, 1:2],
                        scalar1=0.0, scalar2=None, op0=mybir.AluOpType.add,
                    )
                    add_dep_helper(cv.ins, prev_v.ins, reason="DVE WAW")

                a = nc.scalar.activation(
                    out=scr_a[:, r % 2, :],
                    in_=ps[:, 0:4, :],
                    func=mybir.ActivationFunctionType.Relu,
                    accum_out=red[:, 2 * r:2 * r + 1],
                )
                v = nc.vector.tensor_scalar(
                    out=scr_v[:, r % 2, :],
                    in0=ps[:, 4:8, :],
                    scalar1=0.0, scalar2=None,
                    op0=mybir.AluOpType.max, op1=mybir.AluOpType.add,
                    accum_out=red[:, 2 * r + 1:2 * r + 2],
                )
                cons = [a, v]
                prev_a, prev_v = a, v

            # DVE carrier observing ACT's tail so the final reduce carries
            # a single wait (its ACT dep is covered by this predecessor).
            cfin = nc.vector.tensor_scalar(
                out=tiny_v2[:], in0=red[:, 0:1],
                scalar1=0.0, scalar2=None, op0=mybir.AluOpType.add,
            )
            add_dep_helper(cfin.ins, prev_a.ins,
                           reason="reduce observes ACT tail")
            rsum = nc.vector.reduce_sum(outs_t[:], red[:],
                                        axis=mybir.AxisListType.X)
            add_dep_helper(rsum.ins, cfin.ins, reason="rsum after cfin")
            odma = nc.gpsimd.dma_start(out=outs[:], in_=outs_t[:])

            # Pre-observe every proc on SP so the TileContext-exit drain
            # has nothing left to wait on.
            drain_deps = [c.ins for c in cons] + [rsum.ins, odma.ins]
            drain_deps += [m.ins for m in last_mms[-2:]]
            drain_deps += [d.ins for d in lw_dmas]
            drain_deps += [d.ins for grp in x_dmas[-2:] for d in grp]
            for dins in drain_deps:
                dr = nc.sync.drain(fusable=False)
                add_dep_helper(dr.ins, dins, reason="pre-drain observe")

    _NC_CACHE = nc
    return nc


def _bf16_split(a64):
    """Split float64 array into (hi, lo) bf16 so hi+lo ~= a at ~fp32 prec."""
    hi = a64.astype(BF16_NP)
    lo = (a64 - hi.astype(np.float64)).astype(BF16_NP)
    return hi, lo


def kernel(x, W1, b1, W2, b2, W3, b3):
    global LAST_RESULT
    x = np.asarray(x, dtype=np.float32)
    W1 = np.asarray(W1, dtype=np.float32)
    b1 = np.asarray(b1, dtype=np.float32)
    W2 = np.asarray(W2, dtype=np.float32)
    b2 = np.asarray(b2, dtype=np.float32)
    W3 = np.asarray(W3, dtype=np.float32)
    b3 = np.asarray(b3, dtype=np.float32)

    x0, x1 = x[0], x[1]
    mask = x0 != 0.0

    rows_any = mask.any(axis=1)
    cols_any = mask.any(axis=0)
    ridx = np.nonzero(rows_any)[0]
    cidx = np.nonzero(cols_any)[0]
    rmin, rmax = float(ridx[0]), float(ridx[-1])
    cmin, cmax = float(cidx[0]), float(cidx[-1])

    W12 = W1.astype(np.float64) @ W2.astype(np.float64)
    b12 = b1.astype(np.float64) @ W2.astype(np.float64) + b2
    v0, v1, v2, v3 = W12[0], W12[1], W12[2], W12[3]

    nr_all = (np.arange(H, dtype=np.float64) - rmin) / (rmax - rmin)
    nc_all = (np.arange(W, dtype=np.float64) - cmin) / (cmax - cmin)

    # bf16-rounded features (what the device actually multiplies).
    x0b = x0.astype(BF16_NP)
    x1b = x1.astype(BF16_NP)
    ncb = nc_all.astype(BF16_NP)

    v2h, v2l = _bf16_split(v2)
    v3h, v3l = _bf16_split(v3)
    v1h, v1l = _bf16_split(v1)

    nc_prog = _build_bass()
    in_maps = []
    for c in range(N_CORES):
        r0 = c * ROWS_PER_CORE
        x0c = x0b[r0:r0 + ROWS_PER_CORE]          # [128, 1024] bf16
        x1c = x1b[r0:r0 + ROWS_PER_CORE]

        # xd[r, g, k, :] = rhs rows of image row 4r+g
        # (k: x0, x1, x0, x1, nc, 1, nc, 1)
        xdv = np.empty((NROUNDS, 4, 8, 1024), dtype=BF16_NP)
        xr0 = x0c.reshape(NROUNDS, 4, 1024)
        xr1 = x1c.reshape(NROUNDS, 4, 1024)
        xdv[:, :, 0, :] = xr0
        xdv[:, :, 1, :] = xr1
        xdv[:, :, 2, :] = xr0
        xdv[:, :, 3, :] = xr1
        xdv[:, :, 4, :] = ncb
        xdv[:, :, 5, :] = BF16_NP(1.0)
        xdv[:, :, 6, :] = ncb
        xdv[:, :, 7, :] = BF16_NP(1.0)

        btab = (
            b12[:, None]
            + np.outer(v0, nr_all[r0:r0 + ROWS_PER_CORE])
        )                                          # [128 m, 128 rows] f64
        bh, bl = _bf16_split(btab)

        lwtv = np.zeros((4, 8, 32, 128), dtype=BF16_NP)
        for g in range(4):
            lwtv[g, 0, :, :] = v2h
            lwtv[g, 1, :, :] = v3h
            lwtv[g, 2, :, :] = v2l
            lwtv[g, 3, :, :] = v3l
            lwtv[g, 4, :, :] = v1h
            lwtv[g, 6, :, :] = v1l
            for j in range(32):
                l_loc = 4 * j + g
                lwtv[g, 5, j, :] = bh[:, l_loc]
                lwtv[g, 7, j, :] = bl[:, l_loc]
        lwtv = lwtv.reshape(4, 8, 32 * 128)

        in_maps.append({"xd": xdv, "lwt": lwtv})

    res = run_bass_kernel_spmd(
        nc_prog, in_maps, core_ids=list(range(N_CORES)), trace=TRACE
    )
    LAST_RESULT = res

    S = np.zeros(D, dtype=np.float64)
    for c in range(N_CORES):
        S += res.results[c]["outs"][:, 0].astype(np.float64)

    if not mask.all():
        # Subtract the device's contribution of masked (x0==0) pixels,
        # replicating the device arithmetic (bf16 features, split weights).
        v1e = v1h.astype(np.float64) + v1l.astype(np.float64)
        v3e = v3h.astype(np.float64) + v3l.astype(np.float64)
        btab_all = b12[:, None] + np.outer(v0, nr_all)     # [128, H]
        bh_a, bl_a = _bf16_split(btab_all)
        be = bh_a.astype(np.float64) + bl_a.astype(np.float64)
        zr, zc = np.nonzero(~mask)
        hz = (
            np.outer(x1b[zr, zc].astype(np.float64), v3e)
            + np.outer(ncb[zc].astype(np.float64), v1e)
            + be[:, zr].T
        )
        S -= np.maximum(hz, 0.0).sum(axis=0)

    wsum = float(mask.sum())
    out = (S @ W3.astype(np.float64)) / wsum + b3.astype(np.float64)
    return out.astype(np.float32)


# revision 33
# speedup vs baseline: 1.2343x; 1.2343x over previous
"""Trainium2 Bass kernel for nn_Encoder_74182675137046.

Reference computation (per image of 1024x1024 complex pixels):
    feats = [norm_row, norm_col, x0, x1]  per pixel     [N, 4]
    h   = relu((feats @ W1 + b1) @ W2 + b2)             [N, 128]
    out = h @ W3 + b3                                   [N, 128]
    result = (w * out).sum(0) / w.sum()                 [128]
with w = (x0 != 0), and norm_row/col normalized by masked min/max.

Algebraic folding (exact):
    fc1+fc2 fold:  h_pre = feats @ W12 + b12,  W12 = W1@W2, b12 = b1@W2 + b2
    pool/fc3 swap: (w*out).sum = (sum_p w_p*relu(h_pre_p)) @ W3 + w.sum()*b3
So the device only computes S = sum_p relu(h_pre_p)  (a [128] vector per
core); the tiny [128]x[128,128] tail runs on host in float64.

Device design (per core, 128 image rows = 131072 points):
  - bf16 matmuls (1 cycle/col on PE vs ~2.8 for f32r), K=8 rows per
    group: [x0, x1, x0, x1, nc, 1, nc, 1] paired with hi/lo-split
    weights [v2h, v3h, v2l, v3l, v1h, bh, v1l, bl] so effective weights
    carry ~fp32 precision (only feature values are bf16-rounded).  The
    norm_row term + b12 fold into the per-image-row bias column (bh/bl).
  - fp32 PSUM [128, 8, 512] (TRN2 matmul cannot write 16-bit PSUM).
    Round r covers image rows 4r..4r+3: bank b <- (row 4r + b%4,
    col-half b//4); the two col-halves of a row share one lhsT block.
    Four tile_position row groups run concurrently per wave.
  - Consumers per round (4096 els/partition): ScalarE relu+accum on
    banks 0-3 (FD 2048), VectorE max(0,.)+accum on banks 4-7 (FD 2048),
    running concurrently on disjoint banks; 32 rounds.
  - Accumulators land in red[128, 64]; final reduce_sum + DMA returns
    the per-core S vector; the tail (@ W3, /wsum, +b3) runs on host.
"""

import numpy as np
import ml_dtypes

import concourse.bass as bass
import concourse.tile as tile
from concourse import mybir
from concourse.bass_utils import run_bass_kernel_spmd
from concourse.tile_rust import add_dep_helper

H = 1024
W = 1024
D = 128
N_CORES = 8
ROWS_PER_CORE = H // N_CORES          # 128
NROUNDS = 32                          # rounds per core; 4 image rows each
NB = 8                                # PSUM banks (512 fp32 each)

F32 = mybir.dt.float32
BF16 = mybir.dt.bfloat16
BF16_NP = ml_dtypes.bfloat16

TRACE = False
LAST_RESULT = None

_NC_CACHE = None


def _build_bass():
    """Build the SPMD Bass program (same program on all 8 cores)."""
    global _NC_CACHE
    if _NC_CACHE is not None:
        return _NC_CACHE

    nc = bass.Bass()

    xd = nc.dram_tensor("xd", [NROUNDS // 8, 4, 8, 8, 1024], BF16,
                        kind="ExternalInput")
    lwt = nc.dram_tensor("lwt", [4, 8, 32 * 128], BF16,
                         kind="ExternalInput")
    outs = nc.dram_tensor("outs", [128, 1], F32, kind="ExternalOutput")

    with tile.TileContext(nc) as tc:
        with (
            tc.tile_pool(name="singles", bufs=1) as singles,
            tc.tile_pool(name="psall", bufs=1, space="PSUM") as psall,
        ):
            lw_t = singles.tile([128, 32 * 128], BF16)
            rhs_t = singles.tile([128, NROUNDS, 1024], BF16)
            red = singles.tile([128, 2 * NROUNDS], F32)
            outs_t = singles.tile([128, 1], F32)
            tiny_a = singles.tile([128, 1], F32)
            tiny_v = singles.tile([128, 1], F32)
            tiny_v2 = singles.tile([128, 1], F32)
            # Ping-pong SBUF scratch for consumer outputs: avoids the
            # in-place PSUM write's same-engine distance-1 hazard (which
            # costs a second, ISA-illegal sem wait on the consumers).
            scr_a = singles.tile([128, 2, 4 * 512], BF16)
            scr_v = singles.tile([128, 2, 4 * 512], BF16)
            ps = psall.tile([128, NB, 512], F32)

            # DMA out APs must be a contiguous partition block with the
            # free run inside one partition: multi-level partition views
            # get flattened across partition boundaries by the AP
            # optimizer and clobber neighboring tiles.
            lw_dmas = [
                nc.gpsimd.dma_start(out=lw_t[32 * g:32 * g + 8, :],
                                    in_=lwt[g])
                for g in range(4)
            ]
            # rhs rows (x0, x1 duplicated + const rows) on the sync HWDGE
            # queues; one DMA per (8-round block, group) -- the free run
            # (8 slots x 1024) is contiguous within each partition.
            x_dmas = [
                [nc.sync.dma_start(
                    out=rhs_t[32 * g:32 * g + 8, 8 * blk:8 * blk + 8, :],
                    in_=xd[blk, g])
                 for g in range(4)]
                for blk in range(NROUNDS // 8)
            ]

            def obs_mm(cell, src_ap, tp=0):
                """Tiny observer matmul: natural data dep on src_ap's
                producer becomes this instruction's single sem wait; later
                PE instructions with the same dep get it elided.  Writes a
                scratch PSUM cell that the round's real matmul for that
                bank overwrites (start=True) before any consumer reads."""
                return nc.tensor.matmul(
                    cell, src_ap, src_ap,
                    start=True, stop=True, tile_position=(tp, 0),
                )

            def real_mm(r, b):
                g = b % 4
                half = b // 4
                return nc.tensor.matmul(
                    ps[:, b, :],
                    lw_t[32 * g:32 * g + 8, 128 * r:128 * (r + 1)],
                    rhs_t[32 * g:32 * g + 8, r,
                          512 * half:512 * half + 512],
                    start=True, stop=True,
                    tile_position=(32 * g, 0),
                )

            last_mms = None
            cons = []
            prev_a = prev_v = None
            prev_warm = None
            for r in range(NROUNDS):
                # PE-side observers: each carries exactly one cross-engine
                # sem wait (matmul ISA slots allow only one); the real
                # matmuls run wait-free behind them in queue order.  Wave A
                # (banks 0-3, ACT's) is gated only on ACT(r-1); wave B
                # (banks 4-7) only on DVE(r-1), so each engine's banks are
                # refilled while the other engine still drains.
                obs = []
                if r == 0:
                    for g in range(4):
                        obs.append(obs_mm(ps[0:1, 0, 0:1],
                                          lw_t[32 * g:32 * g + 1, 0:1],
                                          tp=32 * g))
                if r % 8 == 0:
                    blk = r // 8
                    for g in range(4):
                        obs.append(obs_mm(ps[0:1, 0, 0:1],
                                          rhs_t[32 * g:32 * g + 1,
                                                8 * blk, 0:1],
                                          tp=32 * g))
                if prev_a is not None:
                    # Reads cons_a(r-1)'s scratch output (bf16 so the obs
                    # matmul stays bf16; fp32 obs interleaved with FWL bf16
                    # matmuls is a known HW-hang combination) -> one ACT
                    # sem wait, which also covers the bank-0 scratch cell's
                    # WAR vs cons_a(r-1).
                    obs.append(obs_mm(ps[0:1, 0, 0:1],
                                      scr_a[0:1, (r - 1) % 2, 0:1]))
                if obs and prev_warm is not None:
                    add_dep_helper(obs[0].ins, prev_warm.ins,
                                   reason="obs after warm")
                for o1, o2 in zip(obs, obs[1:]):
                    add_dep_helper(o2.ins, o1.ins, reason="obs chain")

                mms = []
                for b in (0, 1, 2, 3):
                    mm = real_mm(r, b)
                    if obs:
                        add_dep_helper(mm.ins, obs[-1].ins,
                                       reason="waveA after obs")
                    mms.append(mm)

                if prev_v is not None:
                    ov = obs_mm(ps[0:1, 4, 0:1],
                                scr_v[0:1, (r - 1) % 2, 0:1])
                    add_dep_helper(ov.ins, mms[-1].ins,
                                   reason="obs_v after waveA")
                else:
                    ov = None
                for b in (4, 5, 6, 7):
                    mm = real_mm(r, b)
                    add_dep_helper(mm.ins, (ov or mms[-1]).ins,
                                   reason="waveB after obs_v")
                    mms.append(mm)

                # Keep the PE HAM window busy while consumers drain:
                # dep-free dummy weight loads (no PSUM side effects; the
                # next real matmul reloads its own weights anyway).
                warm = None
                for _ in range(10):
                    w = nc.tensor.ldweights(lw_t[0:8, 0:128])
                    add_dep_helper(w.ins, (warm or mms[-1]).ins,
                                   reason="warm chain")
                    warm = w
                prev_warm = warm
                last_mms = mms

                # Carriers: tiny same-engine ops absorbing the accumulator
                # completion-sem of the previous consumer, so each consumer
                # keeps a single (PE) sem wait.
                if prev_a is not None:
                    ca = nc.scalar.activation(
                        out=tiny_a[:], in_=red[:, 0:1],
                        func=mybir.ActivationFunctionType.Relu,
                    )
                    add_dep_helper(ca.ins, prev_a.ins, reason="ACT WAW")
                if prev_v is not None:
                    cv = nc.vector.tensor_scalar(
                        out=tiny_v[:], in0=red[:, 1:2],
                        scalar1=0.0, scalar2=None, op0=mybir.AluOpType.add,
                    )
                    add_dep_helper(cv.ins, prev_v.ins, reason="DVE WAW")

                a = nc.scalar.activation(
                    out=scr_a[:, r % 2, :],
                    in_=ps[:, 0:4, :],
                    func=mybir.ActivationFunctionType.Relu,
                    accum_out=red[:, 2 * r:2 * r + 1],
                )
                v = nc.vector.tensor_scalar(
                    out=scr_v[:, r % 2, :],
                    in0=ps[:, 4:8, :],
                    scalar1=0.0, scalar2=None,
                    op0=mybir.AluOpType.max, op1=mybir.AluOpType.add,
                    accum_out=red[:, 2 * r + 1:2 * r + 2],
                )
                cons = [a, v]
                prev_a, prev_v = a, v

            # DVE carrier observing ACT's tail so the final reduce carries
            # a single wait (its ACT dep is covered by this predecessor).
            cfin = nc.vector.tensor_scalar(
                out=tiny_v2[:], in0=red[:, 0:1],
                scalar1=0.0, scalar2=None, op0=mybir.AluOpType.add,
            )
            add_dep_helper(cfin.ins, prev_a.ins,
                           reason="reduce observes ACT tail")
            rsum = nc.vector.reduce_sum(outs_t[:], red[:],
                                        axis=mybir.AxisListType.X)
            add_dep_helper(rsum.ins, cfin.ins, reason="rsum after cfin")
            odma = nc.gpsimd.dma_start(out=outs[:], in_=outs_t[:])

            # Pre-observe every proc on SP so the TileContext-exit drain
            # has nothing left to wait on.
            drain_deps = [c.ins for c in cons] + [rsum.ins, odma.ins]
            drain_deps += [m.ins for m in last_mms[-2:]]
            drain_deps += [d.ins for d in lw_dmas]
            drain_deps += [d.ins for grp in x_dmas[-2:] for d in grp]
            for dins in drain_deps:
                dr = nc.sync.drain(fusable=False)
                add_dep_helper(dr.ins, dins, reason="pre-drain observe")

    _NC_CACHE = nc
    return nc


def _bf16_split(a64):
    """Split float64 array into (hi, lo) bf16 so hi+lo ~= a at ~fp32 prec."""
    hi = a64.astype(BF16_NP)
    lo = (a64 - hi.astype(np.float64)).astype(BF16_NP)
    return hi, lo


def kernel(x, W1, b1, W2, b2, W3, b3):
    global LAST_RESULT
    x = np.asarray(x, dtype=np.float32)
    W1 = np.asarray(W1, dtype=np.float32)
    b1 = np.asarray(b1, dtype=np.float32)
    W2 = np.asarray(W2, dtype=np.float32)
    b2 = np.asarray(b2, dtype=np.float32)
    W3 = np.asarray(W3, dtype=np.float32)
    b3 = np.asarray(b3, dtype=np.float32)

    x0, x1 = x[0], x[1]
    mask = x0 != 0.0

    rows_any = mask.any(axis=1)
    cols_any = mask.any(axis=0)
    ridx = np.nonzero(rows_any)[0]
    cidx = np.nonzero(cols_any)[0]
    rmin, rmax = float(ridx[0]), float(ridx[-1])
    cmin, cmax = float(cidx[0]), float(cidx[-1])

    W12 = W1.astype(np.float64) @ W2.astype(np.float64)
    b12 = b1.astype(np.float64) @ W2.astype(np.float64) + b2
    v0, v1, v2, v3 = W12[0], W12[1], W12[2], W12[3]

    nr_all = (np.arange(H, dtype=np.float64) - rmin) / (rmax - rmin)
    nc_all = (np.arange(W, dtype=np.float64) - cmin) / (cmax - cmin)

    # bf16-rounded features (what the device actually multiplies).
    x0b = x0.astype(BF16_NP)
    x1b = x1.astype(BF16_NP)
    ncb = nc_all.astype(BF16_NP)

    v2h, v2l = _bf16_split(v2)
    v3h, v3l = _bf16_split(v3)
    v1h, v1l = _bf16_split(v1)

    nc_prog = _build_bass()
    in_maps = []
    for c in range(N_CORES):
        r0 = c * ROWS_PER_CORE
        x0c = x0b[r0:r0 + ROWS_PER_CORE]          # [128, 1024] bf16
        x1c = x1b[r0:r0 + ROWS_PER_CORE]

        # xd[blk, g, k, s, :] = rhs row k of image row 4*(8*blk+s)+g
        # (k: x0, x1, x0, x1, nc, 1, nc, 1)
        xdv = np.empty((NROUNDS, 4, 8, 1024), dtype=BF16_NP)
        xr0 = x0c.reshape(NROUNDS, 4, 1024)
        xr1 = x1c.reshape(NROUNDS, 4, 1024)
        xdv[:, :, 0, :] = xr0
        xdv[:, :, 1, :] = xr1
        xdv[:, :, 2, :] = xr0
        xdv[:, :, 3, :] = xr1
        xdv[:, :, 4, :] = ncb
        xdv[:, :, 5, :] = BF16_NP(1.0)
        xdv[:, :, 6, :] = ncb
        xdv[:, :, 7, :] = BF16_NP(1.0)
        xdv = np.ascontiguousarray(
            xdv.reshape(NROUNDS // 8, 8, 4, 8, 1024).transpose(0, 2, 3, 1, 4)
        )

        btab = (
            b12[:, None]
            + np.outer(v0, nr_all[r0:r0 + ROWS_PER_CORE])
        )                                          # [128 m, 128 rows] f64
        bh, bl = _bf16_split(btab)

        lwtv = np.zeros((4, 8, 32, 128), dtype=BF16_NP)
        for g in range(4):
            lwtv[g, 0, :, :] = v2h
            lwtv[g, 1, :, :] = v3h
            lwtv[g, 2, :, :] = v2l
            lwtv[g, 3, :, :] = v3l
            lwtv[g, 4, :, :] = v1h
            lwtv[g, 6, :, :] = v1l
            for j in range(32):
                l_loc = 4 * j + g
                lwtv[g, 5, j, :] = bh[:, l_loc]
                lwtv[g, 7, j, :] = bl[:, l_loc]
        lwtv = lwtv.reshape(4, 8, 32 * 128)

        in_maps.append({"xd": xdv, "lwt": lwtv})

    res = run_bass_kernel_spmd(
        nc_prog, in_maps, core_ids=list(range(N_CORES)), trace=TRACE
    )
    LAST_RESULT = res

    S = np.zeros(D, dtype=np.float64)
    for c in range(N_CORES):
        S += res.results[c]["outs"][:, 0].astype(np.float64)

    if not mask.all():
        # Subtract the device's contribution of masked (x0==0) pixels,
        # replicating the device arithmetic (bf16 features, split weights).
        v1e = v1h.astype(np.float64) + v1l.astype(np.float64)
        v3e = v3h.astype(np.float64) + v3l.astype(np.float64)
        btab_all = b12[:, None] + np.outer(v0, nr_all)     # [128, H]
        bh_a, bl_a = _bf16_split(btab_all)
        be = bh_a.astype(np.float64) + bl_a.astype(np.float64)
        zr, zc = np.nonzero(~mask)
        hz = (
            np.outer(x1b[zr, zc].astype(np.float64), v3e)
            + np.outer(ncb[zc].astype(np.float64), v1e)
            + be[:, zr].T
        )
        S -= np.maximum(hz, 0.0).sum(axis=0)

    wsum = float(mask.sum())
    out = (S @ W3.astype(np.float64)) / wsum + b3.astype(np.float64)
    return out.astype(np.float32)
